# revision 2
# baseline (speedup 1.0000x reference)
"""Trainium2 Bass kernel for DiceLoss (hard-argmax dice, ignore background, mean).

Problem (hardcoded shapes):
  y_true: [16, 512, 512] int32 in [0, 8)
  y_pred: [16, 8, 512, 512] float32
  out   : scalar float32 = mean over classes 1..7 of
          (2*tp + eps) / (2*tp + fp + fn + eps)

Strategy v2 (8 NeuronCores, 2 images per core):
  - Single SWDGE cast-DMA FIFO carries EVERYTHING: labels first
    (int32 -> bf16 cast, per image), then y_pred planes (f32 -> bf16).
    One queue keeps the HBM pipe saturated end-to-end; labels arrive
    ~12us in so DVE gt-mask work starts early. No HWDGE label loads, no
    ScalarE int->bf16 converts.
  - DVE: gt masks (TS is_equal 4x) into a unified per-image gt tile
    [P, 7, NSUB, 129] whose col 128 is a ones column (ONE strided
    memset per image); 7-op serial max chain per section; pred masks
    (TT is_equal 2x).
  - TensorE per class c: one psum bank [128, 387]:
      cols 0:129   tp/predcnt: lhsT=pred_c subtile, rhs=gt[c,s,0:129]
                   accumulated over 32 subtiles (col 128 = pred counts)
      cols 129:387 gt counts:  lhsT=ones128, rhs=gt[c, 2 subtiles flat]
                   accumulated over 16 chunks; rows all identical =
                   per-column gt sums. Scheduled early (gt-only deps).
    This replaces the ScalarE flat-copy gt-count path entirely (ScalarE
    was the 2nd-busiest engine in the baseline at ~56us).
  - ScalarE: only psum evacuation (7 x [128, 387] copies).
  - Host: tp = trace, pred_cnt = col 128 sums, gt_cnt = G row sums.
"""

import numpy as np

EPS = 1e-05

N_CORES = 8
NB = 2            # batch images per core
C = 8             # classes
P = 128           # SBUF partitions
FD = 2048         # free-dim elements per channel plane (512*512 = 128*2048)
NSUB = FD // 128  # 128-wide subtiles per plane
BLK = 129         # gt block: 128 gt cols + ones col
GW = 2 * BLK      # G-matmul chunk width (2 subtiles)

_CACHED_NC = None


def build_bass():
    from contextlib import ExitStack

    import concourse.bacc as bacc
    import concourse.tile as tile
    from concourse import mybir

    nc = bacc.Bacc(None, target_bir_lowering=False)

    yp = nc.dram_tensor("yp", [NB, C, P, FD], mybir.dt.float32, kind="ExternalInput")
    yt = nc.dram_tensor("yt", [NB, P, FD], mybir.dt.int32, kind="ExternalInput")
    # per class: [128, 129] tp/predcnt block
    mm_out = nc.dram_tensor("mm_out", [7, P, 129], mybir.dt.float32, kind="ExternalOutput")
    # per class: [128, 258] gt-count block (rows identical), evac'd mid-stream
    g_out = nc.dram_tensor("g_out", [7, P, 258], mybir.dt.float32, kind="ExternalOutput")

    with tile.TileContext(nc) as tc, ExitStack() as ctx:
        chpool = ctx.enter_context(tc.tile_pool(name="ch", bufs=1))
        tpool = ctx.enter_context(tc.tile_pool(name="tt", bufs=1))
        mpool = ctx.enter_context(tc.tile_pool(name="mx", bufs=2))
        mtmp = ctx.enter_context(tc.tile_pool(name="mtmp", bufs=6))
        predp = ctx.enter_context(tc.tile_pool(name="pred", bufs=5))
        accp = ctx.enter_context(tc.tile_pool(name="acc", bufs=1))
        psump = ctx.enter_context(tc.tile_pool(name="psum", bufs=1, space="PSUM"))

        gtsets = [
            accp.tile([P, 7, NSUB, BLK], mybir.dt.bfloat16, name=f"gt{n}")
            for n in range(NB)
        ]
        ones128 = accp.tile([P, 128], mybir.dt.bfloat16, name="ones128")
        psums = [
            psump.tile([P, 387], mybir.dt.float32, name=f"ps{c}", tag=f"ps{c}")
            for c in range(1, C)
        ]

        nc.vector.memset(ones128, 1.0)
        for g in gtsets:
            nc.vector.memset(g[:, :, :, 128:129], 1.0)

        HF = FD // 2
        HS = NSUB // 2

        # ---- all loads up front on the single SWDGE cast queue:
        # labels (per image) first, then img0 planes, then img1 halves ----
        tf = {}
        for n in range(NB):
            tfn = tpool.tile([P, FD], mybir.dt.bfloat16, name="tf", tag=f"tf{n}")
            nc.gpsimd.dma_start(out=tfn, in_=yt[n])
            tf[n] = tfn
        ch = {}
        for c in range(C):
            tl = chpool.tile([P, FD], mybir.dt.bfloat16, name=f"ch{c}", tag=f"n0ch{c}")
            nc.gpsimd.dma_start(out=tl, in_=yp[0, c])
            ch[0, c] = tl
        sections = [(0, HF), (HF, HF)]
        im1 = {}
        for c in range(C):
            im1[c] = chpool.tile([P, FD], mybir.dt.bfloat16, name=f"ch{c}", tag=f"n1ch{c}")
        for si, (off, ln) in enumerate(sections):
            for c in range(C):
                part = im1[c][:, off : off + ln]
                nc.gpsimd.dma_start(out=part, in_=yp[1, c][:, off : off + ln])
                ch[1, c, si] = part

        # Measured SWDGE FIFO timing (ms): first bytes ~9.5us in (labels),
        # then one 1 MiB read every ~2.7us (half planes ~1.35us).
        T0 = 0.0095       # first label-image cast lands
        DT_FULL = 0.0027  # per 1 MiB HBM read
        DT_HALF = 0.00135

        def emit_gt(n, tfv, c, s0, ns, ts):
            g = gtsets[n]
            with tc.tile_wait_until(ts):
                nc.vector.tensor_single_scalar(
                    out=g[:, c - 1, s0 : s0 + ns, 0:128], in_=tfv,
                    scalar=float(c), op=mybir.AluOpType.is_equal,
                )

        def emit_gcnt(n, c, ts):
            """gt-count matmuls for class c, image n: 8 chunks of 2 subtiles."""
            g = gtsets[n]
            for k in range(NSUB // 2):
                rhs = g[:, c - 1, 2 * k : 2 * k + 2, :].rearrange("p s f -> p (s f)")
                with tc.tile_wait_until(ts):
                    nc.tensor.matmul(
                        psums[c - 1][:, 129:387], lhsT=ones128, rhs=rhs,
                        start=(n == 0 and k == 0),
                        stop=(n == NB - 1 and k == NSUB // 2 - 1),
                    )

        def emit_tree(chs, fd, t0, dt):
            t1 = mtmp.tile([P, FD], mybir.dt.bfloat16, name="t1", tag="mt")
            with tc.tile_wait_until(t0 + 2 * dt):
                nc.vector.tensor_max(t1[:, 0:fd], chs[0], chs[1])
            t2 = mtmp.tile([P, FD], mybir.dt.bfloat16, name="t2", tag="mt")
            t12 = mtmp.tile([P, FD], mybir.dt.bfloat16, name="t12", tag="mt")
            with tc.tile_wait_until(t0 + 4 * dt):
                nc.vector.tensor_max(t2[:, 0:fd], chs[2], chs[3])
                nc.vector.tensor_max(t12[:, 0:fd], t1[:, 0:fd], t2[:, 0:fd])
            t3 = mtmp.tile([P, FD], mybir.dt.bfloat16, name="t3", tag="mt")
            t123 = mtmp.tile([P, FD], mybir.dt.bfloat16, name="t123", tag="mt")
            with tc.tile_wait_until(t0 + 6 * dt):
                nc.vector.tensor_max(t3[:, 0:fd], chs[4], chs[5])
                nc.vector.tensor_max(t123[:, 0:fd], t12[:, 0:fd], t3[:, 0:fd])
            t6 = mtmp.tile([P, FD], mybir.dt.bfloat16, name="t6", tag="mt")
            with tc.tile_wait_until(t0 + 7 * dt):
                nc.vector.tensor_max(t6[:, 0:fd], t123[:, 0:fd], chs[6])
            m = mpool.tile([P, FD], mybir.dt.bfloat16, name="m", tag="m")
            with tc.tile_wait_until(t0 + 8 * dt):
                nc.vector.tensor_max(m[:, 0:fd], t6[:, 0:fd], chs[7])
            return m

        def emit_pred_mm(n, chv, m, c, s0, ns, start, stop):
            g = gtsets[n]
            pred = predp.tile([P, FD], mybir.dt.bfloat16, name=f"pred{c}", tag="pred")
            predv = pred[:, 0 : ns * 128]
            nc.vector.tensor_tensor(
                out=predv, in0=chv, in1=m, op=mybir.AluOpType.is_equal
            )
            for s in range(ns):
                nc.tensor.matmul(
                    psums[c - 1][:, 0:129],
                    lhsT=predv[:, s * 128 : (s + 1) * 128],
                    rhs=g[:, c - 1, s0 + s, :],
                    start=(start and s == 0),
                    stop=(stop and s == ns - 1),
                )

        # ---- DVE program ----
        # gt masks per image as its labels land; G matmuls right after
        # (PE is otherwise idle until the first pred masks ~T0+20us).
        tf3 = {n: tf[n].rearrange("p (s f) -> p s f", s=NSUB) for n in range(NB)}
        for n in range(NB):
            ts = T0 + n * DT_FULL
            for c in range(1, C):
                emit_gt(n, tf3[n], c, 0, NSUB, ts)
        for n in range(NB):
            for c in range(1, C):
                emit_gcnt(n, c, T0 + 2 * DT_FULL)

        # img0: planes land T0+2dt .. T0+10dt
        TP = T0 + 2 * DT_FULL
        m0 = emit_tree([ch[0, c] for c in range(C)], FD, TP - DT_FULL, DT_FULL)
        for c in range(1, C):
            emit_pred_mm(0, ch[0, c], m0, c, 0, NSUB, start=True, stop=False)

        for si, (off, ln) in enumerate(sections):
            s0, ns = off // 128, ln // 128
            t0 = TP + 8 * DT_FULL + si * 8 * DT_HALF - DT_HALF
            mh = emit_tree([ch[1, c, si] for c in range(C)], ln, t0, DT_HALF)
            for c in range(1, C):
                emit_pred_mm(
                    1, ch[1, c, si], mh[:, 0:ln],
                    c, s0, ns, start=False, stop=(si == len(sections) - 1),
                )

        # G-count evac: deps only on gt masks, so this lands mid-stream
        for c in range(7):
            gp = accp.tile([P, 258], mybir.dt.float32, name=f"gp{c}", tag=f"gp{c}")
            nc.scalar.copy(out=gp, in_=psums[c][:, 129:387])
            nc.sync.dma_start(out=g_out[c], in_=gp)
        # tp evac: trails the final matmuls
        for c in range(7):
            pt = accp.tile([P, 129], mybir.dt.float32, name=f"pt{c}", tag=f"pt{c}")
            nc.scalar.copy(out=pt, in_=psums[c][:, 0:129])
            nc.sync.dma_start(out=mm_out[c], in_=pt)

    nc.finalize()
    return nc


def _get_bass():
    global _CACHED_NC
    if _CACHED_NC is None:
        _CACHED_NC = build_bass()
    return _CACHED_NC


def make_in_maps(y_true, y_pred):
    yp = np.ascontiguousarray(np.asarray(y_pred, dtype=np.float32))
    yt = np.ascontiguousarray(np.asarray(y_true, dtype=np.int32))
    in_maps = []
    for i in range(N_CORES):
        yps = np.ascontiguousarray(yp[NB * i : NB * (i + 1)]).reshape(NB, C, P, FD)
        yts = np.ascontiguousarray(yt[NB * i : NB * (i + 1)]).reshape(NB, P, FD)
        in_maps.append({"yp": yps, "yt": yts})
    return in_maps


def epilogue(results):
    tp = np.zeros(7, dtype=np.float64)
    pred_cnt = np.zeros(7, dtype=np.float64)
    gt_cnt = np.zeros(7, dtype=np.float64)
    for r in results:
        mm = np.asarray(r["mm_out"], dtype=np.float64)  # [7, P, 129]
        tp += np.trace(mm[:, :, 0:128], axis1=1, axis2=2)
        pred_cnt += mm[:, :, 128].sum(axis=1)
        # G block rows identical; use row 0, skip ones cols 128 & 257
        g = np.asarray(r["g_out"], dtype=np.float64)[:, 0, :]  # [7, 258]
        gt_cnt += g[:, 0:128].sum(axis=1) + g[:, 129:257].sum(axis=1)

    tp32 = tp.astype(np.float32)
    fp32_ = (pred_cnt - tp).astype(np.float32)
    fn32 = (gt_cnt - tp).astype(np.float32)
    eps = np.float32(EPS)
    two = np.float32(2.0)
    dice = (two * tp32 + eps) / (two * tp32 + fp32_ + fn32 + eps)
    return np.asarray(np.mean(dice, dtype=np.float32), dtype=np.float32)


def kernel(**inputs):
    from concourse.bass_utils import run_bass_kernel_spmd

    nc = _get_bass()
    in_maps = make_in_maps(inputs["y_true"], inputs["y_pred"])
    res = run_bass_kernel_spmd(nc, in_maps, core_ids=list(range(N_CORES)))
    return epilogue(res.results)


if __name__ == "__main__":
    rng = np.random.default_rng(0)
    y_true = rng.integers(0, C, size=(16, 512, 512)).astype(np.int32)
    y_pred = rng.standard_normal((16, C, 512, 512)).astype(np.float32)
    out = kernel(y_true=y_true, y_pred=y_pred)
    print("kernel output:", out)


# revision 4
# speedup vs baseline: 1.0083x; 1.0083x over previous
"""Trainium2 Bass kernel for DiceLoss (hard-argmax dice, ignore background, mean).

Problem (hardcoded shapes):
  y_true: [16, 512, 512] int32 in [0, 8)
  y_pred: [16, 8, 512, 512] float32
  out   : scalar float32 = mean over classes 1..7 of
          (2*tp + eps) / (2*tp + fp + fn + eps)

Strategy v2.1 (8 NeuronCores, 2 images per core):
  - ONE SWDGE cast-DMA FIFO, 14 transfers total: labels per image
    (int32 -> bf16), then channel-PAIR transfers (two planes per DMA,
    f32 -> bf16). Few big DMAs means all Q7 descriptor generation
    finishes by ~30us -- before DVE goes dense with perf-mode ops whose
    shared-port locks starve SWDGE desc-gen (v2's stream collapsed to
    12 GB/s in its last 8us because of exactly that).
  - Pair tiles match the max-tree's first level: pmax_k = max(ch2k,
    ch2k+1) fires the moment pair k lands; a serial chain follows.
  - DVE: gt masks (TS is_equal 4x) into unified per-image gt tiles
    [P, 7, NSUB, 129] (col 128 = ones, ONE strided memset per image);
    tree + pred masks (TT 2x).
  - TensorE: per class, tp/predcnt matmuls (lhsT=pred subtile,
    rhs=gt[c,s,0:129], 32 accumulations; col 128 = pred counts) into
    role-split psum tiles T_A(c1-3)/T_B(c4-6)/T_C(c7); gt-count
    matmuls (lhsT=ones128, rhs=gt[c,s,:], 32 accumulations, all rows
    identical = per-column gt sums) into G_A/G_B/G_C. The role split
    matters: psum deps are tile-granular, so G evacs only wait on G
    matmuls (label-dependent only) and fly out mid-stream.
  - ScalarE: psum evacuation only (baseline had 42us of flat copies).
  - Host: tp = trace, pred_cnt = col-128 sums, gt_cnt = G row-0 sums.
"""

import numpy as np

EPS = 1e-05

N_CORES = 8
NB = 2            # batch images per core
C = 8             # classes
P = 128           # SBUF partitions
FD = 2048         # free-dim elements per channel plane (512*512 = 128*2048)
NSUB = FD // 128  # 128-wide subtiles per plane
HF = FD // 2      # half-plane section length
HS = NSUB // 2

_CACHED_NC = None


def build_bass():
    from contextlib import ExitStack

    import concourse.bacc as bacc
    import concourse.tile as tile
    from concourse import mybir

    nc = bacc.Bacc(None, target_bir_lowering=False)

    yp = nc.dram_tensor("yp", [NB, C, P, FD], mybir.dt.float32, kind="ExternalInput")
    yt = nc.dram_tensor("yt", [NB, P, FD], mybir.dt.int32, kind="ExternalInput")
    # cols: class blocks of 129 in order c1..c7: [c1-3 | c4-6 | c7]
    mm_out = nc.dram_tensor("mm_out", [P, 903], mybir.dt.float32, kind="ExternalOutput")
    g_out = nc.dram_tensor("g_out", [P, 903], mybir.dt.float32, kind="ExternalOutput")

    with tile.TileContext(nc) as tc, ExitStack() as ctx:
        chpool = ctx.enter_context(tc.tile_pool(name="ch", bufs=1))
        tpool = ctx.enter_context(tc.tile_pool(name="tt", bufs=1))
        mpool = ctx.enter_context(tc.tile_pool(name="mx", bufs=2))
        mtmp = ctx.enter_context(tc.tile_pool(name="mtmp", bufs=6))
        predp = ctx.enter_context(tc.tile_pool(name="pred", bufs=5))
        accp = ctx.enter_context(tc.tile_pool(name="acc", bufs=1))
        psump = ctx.enter_context(tc.tile_pool(name="psum", bufs=1, space="PSUM"))

        gtsets = [
            accp.tile([P, 7, NSUB, 129], mybir.dt.bfloat16, name=f"gt{n}")
            for n in range(NB)
        ]
        ones128 = accp.tile([P, 128], mybir.dt.bfloat16, name="ones128")
        tpA = psump.tile([P, 387], mybir.dt.float32, name="tpA", tag="tpA")
        tpB = psump.tile([P, 387], mybir.dt.float32, name="tpB", tag="tpB")
        tpC = psump.tile([P, 129], mybir.dt.float32, name="tpC", tag="tpC")
        gA = psump.tile([P, 387], mybir.dt.float32, name="gA", tag="gA")
        gB = psump.tile([P, 387], mybir.dt.float32, name="gB", tag="gB")
        gC = psump.tile([P, 129], mybir.dt.float32, name="gC", tag="gC")

        def tp_ap(c):
            t, off = [(tpA, 0), (tpA, 129), (tpA, 258), (tpB, 0), (tpB, 129),
                      (tpB, 258), (tpC, 0)][c - 1]
            return t[:, off : off + 129]

        def g_ap(c):
            t, off = [(gA, 0), (gA, 129), (gA, 258), (gB, 0), (gB, 129),
                      (gB, 258), (gC, 0)][c - 1]
            return t[:, off : off + 129]

        nc.vector.memset(ones128, 1.0)
        for g in gtsets:
            nc.vector.memset(g[:, :, :, 128:129], 1.0)

        # ---- the single SWDGE FIFO: labels, img0 pairs, img1 section
        # pairs. Measured: first bytes ~9.5us; ~2.7us per MiB HBM read. ----
        T_TF = [0.0121, 0.0148]
        T_P0 = [0.0202, 0.0256, 0.0310, 0.0364]   # img0 pair k lands
        T_S = [[0.0391, 0.0418, 0.0445, 0.0472],  # img1 sec0 pair k
               [0.0499, 0.0526, 0.0553, 0.0580]]  # img1 sec1 pair k

        tf = {}
        for n in range(NB):
            tfn = tpool.tile([P, FD], mybir.dt.bfloat16, name="tf", tag=f"tf{n}")
            nc.gpsimd.dma_start(out=tfn, in_=yt[n])
            tf[n] = tfn
        pairs0 = []
        for k in range(4):
            pt = chpool.tile([P, 2, FD], mybir.dt.bfloat16, name=f"p0_{k}", tag=f"p0_{k}")
            src = yp[0, 2 * k : 2 * k + 2].rearrange("c p x -> p c x")
            nc.gpsimd.dma_start(out=pt, in_=src)
            pairs0.append(pt)
        pairs1 = [[], []]
        for si in range(2):
            off = si * HF
            for k in range(4):
                pt = chpool.tile([P, 2, HF], mybir.dt.bfloat16,
                                 name=f"p1_{si}_{k}", tag=f"p1_{si}_{k}")
                src = yp[1, 2 * k : 2 * k + 2, :, off : off + HF].rearrange(
                    "c p x -> p c x")
                nc.gpsimd.dma_start(out=pt, in_=src)
                pairs1[si].append(pt)

        def ch0(c):
            return pairs0[c // 2][:, c % 2, :]

        def ch1(si, c):
            return pairs1[si][c // 2][:, c % 2, :]

        def emit_gt(n, c, ts):
            tf3 = tf[n].rearrange("p (s f) -> p s f", s=NSUB)
            with tc.tile_wait_until(ts):
                nc.vector.tensor_single_scalar(
                    out=gtsets[n][:, c - 1, :, 0:128], in_=tf3,
                    scalar=float(c), op=mybir.AluOpType.is_equal,
                )

        def emit_gcnt(n, c, ts):
            g = gtsets[n]
            for s in range(NSUB):
                with tc.tile_wait_until(ts):
                    nc.tensor.matmul(
                        g_ap(c), lhsT=ones128, rhs=g[:, c - 1, s, :],
                        start=(n == 0 and s == 0),
                        stop=(n == NB - 1 and s == NSUB - 1),
                    )

        def emit_tree(chs, fd, gates):
            """chs[k] = pair k's two channel APs; gates[k] = arrival stamp."""
            pm = []
            for k in range(4):
                t = mtmp.tile([P, FD], mybir.dt.bfloat16, name=f"pm{k}", tag="mt")
                with tc.tile_wait_until(gates[k]):
                    nc.vector.tensor_max(t[:, 0:fd], chs[k][0], chs[k][1])
                pm.append(t[:, 0:fd])
            r1 = mtmp.tile([P, FD], mybir.dt.bfloat16, name="r1", tag="mt")
            with tc.tile_wait_until(gates[1]):
                nc.vector.tensor_max(r1[:, 0:fd], pm[0], pm[1])
            r2 = mtmp.tile([P, FD], mybir.dt.bfloat16, name="r2", tag="mt")
            with tc.tile_wait_until(gates[2]):
                nc.vector.tensor_max(r2[:, 0:fd], r1[:, 0:fd], pm[2])
            m = mpool.tile([P, FD], mybir.dt.bfloat16, name="m", tag="m")
            with tc.tile_wait_until(gates[3]):
                nc.vector.tensor_max(m[:, 0:fd], r2[:, 0:fd], pm[3])
            return m

        def emit_pred_mm(n, chv, m, c, s0, ns, start, stop, ts):
            g = gtsets[n]
            pred = predp.tile([P, FD], mybir.dt.bfloat16, name=f"pred{c}", tag="pred")
            predv = pred[:, 0 : ns * 128]
            with tc.tile_wait_until(ts):
                nc.vector.tensor_tensor(
                    out=predv, in0=chv, in1=m, op=mybir.AluOpType.is_equal
                )
            for s in range(ns):
                nc.tensor.matmul(
                    tp_ap(c),
                    lhsT=predv[:, s * 128 : (s + 1) * 128],
                    rhs=g[:, c - 1, s0 + s, :],
                    start=(start and s == 0),
                    stop=(stop and s == ns - 1),
                )

        # gt masks as each label image lands; G matmuls right after.
        for n in range(NB):
            for c in range(1, C):
                emit_gt(n, c, T_TF[n])
        for n in range(NB):
            for c in range(1, C):
                emit_gcnt(n, c, T_TF[1] + 0.002)

        # G evac mid-stream (deps: G matmuls only, which need just labels)
        for name, gt_, width, off in (("gA", gA, 387, 0), ("gB", gB, 387, 387),
                                      ("gC", gC, 129, 774)):
            ev = accp.tile([P, width], mybir.dt.float32, name=f"ev{name}", tag=f"ev{name}")
            nc.scalar.copy(out=ev, in_=gt_)
            nc.sync.dma_start(out=g_out[:, off : off + width], in_=ev)

        # img0: tree + preds
        m0 = emit_tree([(ch0(2 * k), ch0(2 * k + 1)) for k in range(4)], FD, T_P0)
        for c in range(1, C):
            emit_pred_mm(0, ch0(c), m0, c, 0, NSUB, start=True, stop=False,
                         ts=T_P0[3] + 0.0011)

        # img1 sections
        for si in range(2):
            s0 = si * HS
            mh = emit_tree(
                [(ch1(si, 2 * k), ch1(si, 2 * k + 1)) for k in range(4)], HF, T_S[si])
            for c in range(1, C):
                emit_pred_mm(1, ch1(si, c), mh[:, 0:HF],
                             c, s0, HS, start=False, stop=(si == 1),
                             ts=T_S[si][3] + 0.0006)

        # tp evac: c7 in its own small tile so the last chain is short
        for name, t_, width, off in (("tpA", tpA, 387, 0), ("tpB", tpB, 387, 387),
                                     ("tpC", tpC, 129, 774)):
            ev = accp.tile([P, width], mybir.dt.float32, name=f"ev{name}", tag=f"ev{name}")
            nc.scalar.copy(out=ev, in_=t_)
            nc.sync.dma_start(out=mm_out[:, off : off + width], in_=ev)

    nc.finalize()
    return nc


def _get_bass():
    global _CACHED_NC
    if _CACHED_NC is None:
        _CACHED_NC = build_bass()
    return _CACHED_NC


def make_in_maps(y_true, y_pred):
    yp = np.ascontiguousarray(np.asarray(y_pred, dtype=np.float32))
    yt = np.ascontiguousarray(np.asarray(y_true, dtype=np.int32))
    in_maps = []
    for i in range(N_CORES):
        yps = np.ascontiguousarray(yp[NB * i : NB * (i + 1)]).reshape(NB, C, P, FD)
        yts = np.ascontiguousarray(yt[NB * i : NB * (i + 1)]).reshape(NB, P, FD)
        in_maps.append({"yp": yps, "yt": yts})
    return in_maps


def epilogue(results):
    tp = np.zeros(7, dtype=np.float64)
    pred_cnt = np.zeros(7, dtype=np.float64)
    gt_cnt = np.zeros(7, dtype=np.float64)
    for r in results:
        mm = np.asarray(r["mm_out"], dtype=np.float64)  # [P, 903]
        gg = np.asarray(r["g_out"], dtype=np.float64)   # [P, 903]
        for c in range(7):
            blk = mm[:, 129 * c : 129 * c + 129]
            tp[c] += np.trace(blk[:, 0:128])
            pred_cnt[c] += blk[:, 128].sum()
            # G rows identical; row 0, skip ones col 128
            gt_cnt[c] += gg[0, 129 * c : 129 * c + 128].sum()

    tp32 = tp.astype(np.float32)
    fp32_ = (pred_cnt - tp).astype(np.float32)
    fn32 = (gt_cnt - tp).astype(np.float32)
    eps = np.float32(EPS)
    two = np.float32(2.0)
    dice = (two * tp32 + eps) / (two * tp32 + fp32_ + fn32 + eps)
    return np.asarray(np.mean(dice, dtype=np.float32), dtype=np.float32)


def kernel(**inputs):
    from concourse.bass_utils import run_bass_kernel_spmd

    nc = _get_bass()
    in_maps = make_in_maps(inputs["y_true"], inputs["y_pred"])
    res = run_bass_kernel_spmd(nc, in_maps, core_ids=list(range(N_CORES)))
    return epilogue(res.results)


if __name__ == "__main__":
    rng = np.random.default_rng(0)
    y_true = rng.integers(0, C, size=(16, 512, 512)).astype(np.int32)
    y_pred = rng.standard_normal((16, C, 512, 512)).astype(np.float32)
    out = kernel(y_true=y_true, y_pred=y_pred)
    print("kernel output:", out)


# revision 5
# speedup vs baseline: 1.0733x; 1.0645x over previous
"""Trainium2 Bass kernel for DiceLoss (hard-argmax dice, ignore background, mean).

Problem (hardcoded shapes):
  y_true: [16, 512, 512] int32 in [0, 8)
  y_pred: [16, 8, 512, 512] float32
  out   : scalar float32 = mean over classes 1..7 of
          (2*tp + eps) / (2*tp + fp + fn + eps)

Strategy v2.2 (8 NeuronCores, 2 images per core):
  - ONE SWDGE cast-DMA FIFO: labels per image (int32 -> bf16), then
    channel-PAIR transfers (two planes per DMA, f32 -> bf16). Pair
    tiles match the max-tree's first level.
  - Six tiny dummy SWDGE transfers sit between img0's first and second
    pair: Tile recycles 8 DMA-completion semaphores, so transfer #k's
    descriptor generation blocks until #(k-8)'s DATA lands. Without
    the dummies the last pairs' desc-gen happens >45us -- inside the
    window where DVE perf-mode ops hold the shared SBUF port and
    starve Q7 descriptor writes (v2.1's stream collapsed to 12 GB/s
    for its last 10us). With them, every real transfer's desc-gen
    clears by ~36us.
  - DVE: gt masks (TS is_equal 4x) into ONE gt tile [P, 2, 7, NSUB,
    129] (col 128 = ones, one strided memset); tree + preds (TT 2x).
  - TensorE per class: psum bank [P, 387] = [G 0:258 | tp 258:387].
    G: lhsT=ones128, rhs=gt[:, both-images, c, s, :] (258 cols), 16
    accumulations; rows identical = per-column gt sums. tp/predcnt:
    lhsT=pred subtile, rhs=gt[n, c, s, 0:129], 32 accumulations.
  - ScalarE: evacs only. G evac is row 0 only ([1, 258] per class,
    ~1KB) so the mid-stream g_out DMA can't steal HBM bandwidth
    (v2.1's 790KB g_out delayed img0's last pair by ~8us).
  - Host: tp = trace, pred_cnt = col-128 sums, gt_cnt = G row-0 sums.
"""

import numpy as np

EPS = 1e-05

N_CORES = 8
NB = 2            # batch images per core
C = 8             # classes
P = 128           # SBUF partitions
FD = 2048         # free-dim elements per channel plane (512*512 = 128*2048)
NSUB = FD // 128  # 128-wide subtiles per plane
HF = FD // 2      # half-plane section length
HS = NSUB // 2

_CACHED_NC = None


def build_bass():
    from contextlib import ExitStack

    import concourse.bacc as bacc
    import concourse.tile as tile
    from concourse import mybir

    nc = bacc.Bacc(None, target_bir_lowering=False)

    yp = nc.dram_tensor("yp", [NB, C, P, FD], mybir.dt.float32, kind="ExternalInput")
    yt = nc.dram_tensor("yt", [NB, P, FD], mybir.dt.int32, kind="ExternalInput")
    mm_out = nc.dram_tensor("mm_out", [7, P, 129], mybir.dt.float32, kind="ExternalOutput")
    g_out = nc.dram_tensor("g_out", [7, 258], mybir.dt.float32, kind="ExternalOutput")

    with tile.TileContext(nc) as tc, ExitStack() as ctx:
        chpool = ctx.enter_context(tc.tile_pool(name="ch", bufs=1))
        tpool = ctx.enter_context(tc.tile_pool(name="tt", bufs=1))
        mpool = ctx.enter_context(tc.tile_pool(name="mx", bufs=2))
        mtmp = ctx.enter_context(tc.tile_pool(name="mtmp", bufs=6))
        predp = ctx.enter_context(tc.tile_pool(name="pred", bufs=5))
        accp = ctx.enter_context(tc.tile_pool(name="acc", bufs=1))
        psump = ctx.enter_context(tc.tile_pool(name="psum", bufs=1, space="PSUM"))

        # one gt tile, both images: [P, img, class, subtile, 129]
        gtall = accp.tile([P, NB, 7, NSUB, 129], mybir.dt.bfloat16, name="gtall")
        ones128 = accp.tile([P, 128], mybir.dt.bfloat16, name="ones128")
        psums = [
            psump.tile([P, 387], mybir.dt.float32, name=f"ps{c}", tag=f"ps{c}")
            for c in range(1, C)
        ]

        def g_ap(c):
            return psums[c - 1][:, 0:258]

        def tp_ap(c):
            return psums[c - 1][:, 258:387]

        nc.vector.memset(ones128, 1.0)
        nc.vector.memset(gtall[:, :, :, :, 128:129], 1.0)

        # ---- the single SWDGE FIFO. Measured: first bytes ~9.2us;
        # ~2.6us per MiB of HBM read. ----
        T_TF = [0.0120, 0.0147]
        T_P0 = [0.0201, 0.0253, 0.0305, 0.0357]
        T_S = [[0.0383, 0.0409, 0.0435, 0.0461],
               [0.0487, 0.0513, 0.0539, 0.0565]]

        tf = {}
        for n in range(NB):
            tfn = tpool.tile([P, FD], mybir.dt.bfloat16, name="tf", tag=f"tf{n}")
            nc.gpsimd.dma_start(out=tfn, in_=yt[n])
            tf[n] = tfn
        pairs0 = []
        for k in range(4):
            pt = chpool.tile([P, 2, FD], mybir.dt.bfloat16, name=f"p0_{k}", tag=f"p0_{k}")
            src = yp[0, 2 * k : 2 * k + 2].rearrange("c p x -> p c x")
            nc.gpsimd.dma_start(out=pt, in_=src)
            pairs0.append(pt)
            if k == 1:
                # sem-lane absorbers (see module docstring)
                for j in range(6):
                    dz = tpool.tile([1, 16], mybir.dt.int32, name=f"dz{j}", tag=f"dz{j}")
                    nc.gpsimd.dma_start(out=dz, in_=yt[0][0:1, 0:16])
        pairs1 = [[], []]
        for si in range(2):
            off = si * HF
            for k in range(4):
                pt = chpool.tile([P, 2, HF], mybir.dt.bfloat16,
                                 name=f"p1_{si}_{k}", tag=f"p1_{si}_{k}")
                src = yp[1, 2 * k : 2 * k + 2, :, off : off + HF].rearrange(
                    "c p x -> p c x")
                nc.gpsimd.dma_start(out=pt, in_=src)
                pairs1[si].append(pt)

        def ch0(c):
            return pairs0[c // 2][:, c % 2, :]

        def ch1(si, c):
            return pairs1[si][c // 2][:, c % 2, :]

        def emit_gt(n, c, ts):
            tf3 = tf[n].rearrange("p (s f) -> p s f", s=NSUB)
            with tc.tile_wait_until(ts):
                nc.vector.tensor_single_scalar(
                    out=gtall[:, n, c - 1, :, 0:128], in_=tf3,
                    scalar=float(c), op=mybir.AluOpType.is_equal,
                )

        def emit_gcnt(c, ts):
            for s in range(NSUB):
                with tc.tile_wait_until(ts):
                    nc.tensor.matmul(
                        g_ap(c), lhsT=ones128, rhs=gtall[:, :, c - 1, s, :],
                        start=(s == 0), stop=(s == NSUB - 1),
                    )

        def emit_tree(chs, fd, gates):
            """chs[k] = pair k's two channel APs; gates[k] = arrival stamp."""
            pm = []
            for k in range(4):
                t = mtmp.tile([P, FD], mybir.dt.bfloat16, name=f"pm{k}", tag="mt")
                with tc.tile_wait_until(gates[k]):
                    nc.vector.tensor_max(t[:, 0:fd], chs[k][0], chs[k][1])
                pm.append(t[:, 0:fd])
            r1 = mtmp.tile([P, FD], mybir.dt.bfloat16, name="r1", tag="mt")
            with tc.tile_wait_until(gates[1]):
                nc.vector.tensor_max(r1[:, 0:fd], pm[0], pm[1])
            r2 = mtmp.tile([P, FD], mybir.dt.bfloat16, name="r2", tag="mt")
            with tc.tile_wait_until(gates[2]):
                nc.vector.tensor_max(r2[:, 0:fd], r1[:, 0:fd], pm[2])
            m = mpool.tile([P, FD], mybir.dt.bfloat16, name="m", tag="m")
            with tc.tile_wait_until(gates[3]):
                nc.vector.tensor_max(m[:, 0:fd], r2[:, 0:fd], pm[3])
            return m

        def emit_pred_mm(n, chv, m, c, s0, ns, start, stop, ts):
            pred = predp.tile([P, FD], mybir.dt.bfloat16, name=f"pred{c}", tag="pred")
            predv = pred[:, 0 : ns * 128]
            with tc.tile_wait_until(ts):
                nc.vector.tensor_tensor(
                    out=predv, in0=chv, in1=m, op=mybir.AluOpType.is_equal
                )
            for s in range(ns):
                nc.tensor.matmul(
                    tp_ap(c),
                    lhsT=predv[:, s * 128 : (s + 1) * 128],
                    rhs=gtall[:, n, c - 1, s0 + s, :],
                    start=(start and s == 0),
                    stop=(stop and s == ns - 1),
                )

        for n in range(NB):
            for c in range(1, C):
                emit_gt(n, c, T_TF[n])
        for c in range(1, C):
            emit_gcnt(c, T_TF[1] + 0.002)

        # G evac: row 0 only, mid-stream
        for c in range(1, C):
            ev = accp.tile([1, 258], mybir.dt.float32, name=f"evg{c}", tag=f"evg{c}")
            nc.scalar.copy(out=ev, in_=psums[c - 1][0:1, 0:258])
            nc.sync.dma_start(out=g_out[c - 1], in_=ev)

        m0 = emit_tree([(ch0(2 * k), ch0(2 * k + 1)) for k in range(4)], FD, T_P0)
        for c in range(1, C):
            emit_pred_mm(0, ch0(c), m0, c, 0, NSUB, start=True, stop=False,
                         ts=T_P0[3] + 0.0011)

        for si in range(2):
            s0 = si * HS
            mh = emit_tree(
                [(ch1(si, 2 * k), ch1(si, 2 * k + 1)) for k in range(4)], HF, T_S[si])
            for c in range(1, C):
                emit_pred_mm(1, ch1(si, c), mh[:, 0:HF],
                             c, s0, HS, start=False, stop=(si == 1),
                             ts=T_S[si][3] + 0.0006)

        for c in range(1, C):
            pt = accp.tile([P, 129], mybir.dt.float32, name=f"pt{c}", tag=f"pt{c}")
            nc.scalar.copy(out=pt, in_=psums[c - 1][:, 258:387])
            nc.sync.dma_start(out=mm_out[c - 1], in_=pt)

    nc.finalize()
    return nc


def _get_bass():
    global _CACHED_NC
    if _CACHED_NC is None:
        _CACHED_NC = build_bass()
    return _CACHED_NC


def make_in_maps(y_true, y_pred):
    yp = np.ascontiguousarray(np.asarray(y_pred, dtype=np.float32))
    yt = np.ascontiguousarray(np.asarray(y_true, dtype=np.int32))
    in_maps = []
    for i in range(N_CORES):
        yps = np.ascontiguousarray(yp[NB * i : NB * (i + 1)]).reshape(NB, C, P, FD)
        yts = np.ascontiguousarray(yt[NB * i : NB * (i + 1)]).reshape(NB, P, FD)
        in_maps.append({"yp": yps, "yt": yts})
    return in_maps


def epilogue(results):
    tp = np.zeros(7, dtype=np.float64)
    pred_cnt = np.zeros(7, dtype=np.float64)
    gt_cnt = np.zeros(7, dtype=np.float64)
    for r in results:
        mm = np.asarray(r["mm_out"], dtype=np.float64)  # [7, P, 129]
        tp += np.trace(mm[:, :, 0:128], axis1=1, axis2=2)
        pred_cnt += mm[:, :, 128].sum(axis=1)
        g = np.asarray(r["g_out"], dtype=np.float64)    # [7, 258]
        # cols 0:128 img-slot-0 gt sums, col 128 ones; 129:257 slot-1, 257 ones
        gt_cnt += g[:, 0:128].sum(axis=1) + g[:, 129:257].sum(axis=1)

    tp32 = tp.astype(np.float32)
    fp32_ = (pred_cnt - tp).astype(np.float32)
    fn32 = (gt_cnt - tp).astype(np.float32)
    eps = np.float32(EPS)
    two = np.float32(2.0)
    dice = (two * tp32 + eps) / (two * tp32 + fp32_ + fn32 + eps)
    return np.asarray(np.mean(dice, dtype=np.float32), dtype=np.float32)


def kernel(**inputs):
    from concourse.bass_utils import run_bass_kernel_spmd

    nc = _get_bass()
    in_maps = make_in_maps(inputs["y_true"], inputs["y_pred"])
    res = run_bass_kernel_spmd(nc, in_maps, core_ids=list(range(N_CORES)))
    return epilogue(res.results)


if __name__ == "__main__":
    rng = np.random.default_rng(0)
    y_true = rng.integers(0, C, size=(16, 512, 512)).astype(np.int32)
    y_pred = rng.standard_normal((16, C, 512, 512)).astype(np.float32)
    out = kernel(y_true=y_true, y_pred=y_pred)
    print("kernel output:", out)
